# revision 1
# baseline (speedup 1.0000x reference)
"""Trainium2 Bass kernel for nn_Attribution (sparse local-window attention).

Data-parallel over batch n=8 -> one batch element per NeuronCore.

Per-core computation (c_in=256, ch=128, 64x64 image):
    h    = W1 @ x + b1
    corr = 5x5 local window correlation of h (zero padded), /sqrt(128)
    attn = softmax over the 25 window entries
    samp = sum_k attn_k * shift_k(h)
    gate = sigmoid(relu(W2 @ h + b2)) = 0.5 + 0.5*relu(tanh((z+b2)/2))
    out  = Wout @ (gate * samp) + bout

Layout: positions flattened row-major with 2 zero-pad rows top/bottom
(68 rows x 64 = 4352 positions = 34 chunks of 128).  Scores are computed
"born transposed" (keys of one chunk on partitions, queries on the free
axis): for key chunk c the queries of all subs needing it are contiguous,
so one matmul (n<=384) produces all scores of that chunk.  Out-of-window
entries are killed by a {0,1} band mask after exp; out-of-image x
neighbors are accounted by a denominator correction D (they contribute
exp(0)=1 in the zero-padded reference).  Softmax normalization is
commuted through the output convolution; reciprocals are computed with a
bit-hack seed + 3 Newton iterations on VectorE in a compact (32,128)
layout reached via a 16KB DMA reshape.
"""
import sys

sys.path.insert(0, "/opt/trn_rl_repo")

import numpy as np
import ml_dtypes

import concourse.bass as bass
import concourse.mybir as mybir
import concourse.tile as tile
from concourse import bacc
from concourse.bass_utils import run_bass_kernel_spmd
from concourse.masks import make_identity

F32 = mybir.dt.float32
BF16 = mybir.dt.bfloat16
I32 = mybir.dt.int32
F32R = mybir.dt.float32r
AF = mybir.ActivationFunctionType
ALU = mybir.AluOpType

N, CIN, CH, H, W = 8, 256, 128, 64, 64
HW = H * W                      # 4096
RAD = 2
KROWS = H + 2 * RAD             # 68 padded rows
PADPOS = KROWS * W              # 4352
NCHUNK = PADPOS // 128          # 34 key chunks (2 rows each)
NSUB = H // 2                   # 32 query subs (128 queries each)
SCALE = 1.0 / np.sqrt(np.float32(CH))
RECIP_MAGIC = 0x7EF127EA


def _build_mask_and_D():
    """maskC: (128, 384) {0,1}; col 128*a+q is the score of key (chunk c,
    pos p) vs query q of sub s = c-2+a.  Valid iff |2-2a + p//64 - q//64|
    <= 2 and |p%64 - q%64| <= 2.   D: (32,128) = 5*cnt(qx)."""
    m = np.zeros((128, 384), dtype=np.float32)
    for a in range(3):
        for p in range(128):
            for q in range(128):
                dy = 2 - 2 * a + p // 64 - q // 64
                if abs(dy) <= RAD and abs(p % 64 - q % 64) <= RAD:
                    m[p, 128 * a + q] = 1.0
    maskC = m.astype(ml_dtypes.bfloat16)

    cnt = np.array([sum(1 for dx in range(-RAD, RAD + 1) if not 0 <= qx + dx < W)
                    for qx in range(W)], dtype=np.float32)
    Drow = 5.0 * np.concatenate([cnt, cnt])
    D = np.zeros((64, 128), np.float32)
    D[0:16] = Drow[None, :]
    D[32:48] = Drow[None, :]
    return maskC, D


def build_nc(repeat=1, sim_safe=False):
    nc = bacc.Bacc("TRN2", target_bir_lowering=False, debug=False, num_devices=8)

    x_d = nc.declare_dram_parameter("x", [CIN, HW], BF16, isOutput=False)
    w1t_d = nc.declare_dram_parameter("W1T", [CIN, CH], BF16, isOutput=False)
    b1_d = nc.declare_dram_parameter("b1", [CH, 1], F32, isOutput=False)
    w2t_d = nc.declare_dram_parameter("W2T", [CH, CH], BF16, isOutput=False)
    b2h_d = nc.declare_dram_parameter("b2h", [CH, 1], F32, isOutput=False)
    wot_d = nc.declare_dram_parameter("WoutT", [CH, CIN], BF16, isOutput=False)
    bout_d = nc.declare_dram_parameter("bout2", [CH, 2], F32, isOutput=False)
    boutr_d = nc.declare_dram_parameter("boutrow", [1, CIN], BF16, isOutput=False)
    mask_d = nc.declare_dram_parameter("maskC", [128, 384], BF16, isOutput=False)
    dvec_d = nc.declare_dram_parameter("Dvec", [64, 128], F32, isOutput=False)
    ident_d = nc.declare_dram_parameter("ident", [128, 128], BF16, isOutput=False)
    onescol_d = nc.declare_dram_parameter("onescol_c", [128, 1], BF16, isOutput=False)
    ones1_d = nc.declare_dram_parameter("ones1_c", [1, 512], BF16, isOutput=False)
    out_d = nc.declare_dram_parameter("out", [CIN, HW], F32, isOutput=True)

    with tile.TileContext(nc) as tc:
        with (
            tc.tile_pool(name="per", bufs=1) as per,
            tc.tile_pool(name="xb", bufs=4) as xbp,
            tc.tile_pool(name="sm", bufs=4) as smp,
            tc.tile_pool(name="ot", bufs=4) as otp,
            tc.tile_pool(name="psc", bufs=2, space="PSUM") as psc,   # score chunks
            tc.tile_pool(name="pss", bufs=3, space="PSUM") as pss,   # generic 1-bank
            tc.tile_pool(name="psd", bufs=1, space="PSUM") as psd,   # denominators
        ):
            hpad = per.tile([128, PADPOS], BF16, tag="hpad")
            hT = per.tile([128, PADPOS], BF16, tag="hT")
            attnm = per.tile([128, NCHUNK * 512], BF16, tag="attnm")
            Pg = per.tile([128, HW], BF16, tag="Pg")
            attr = per.tile([128, HW], BF16, tag="attr")
            denrow = per.tile([1, HW], BF16, tag="denrow")
            recrow = per.tile([1, HW], BF16, tag="recrow")

            w1t0 = per.tile([128, CH], BF16, tag="w1t0")
            w1t1 = per.tile([128, CH], BF16, tag="w1t1")
            w2t = per.tile([128, CH], BF16, tag="w2t")
            wot = per.tile([128, CIN], BF16, tag="wot")
            b1 = per.tile([CH, 1], F32, tag="b1")
            b2h = per.tile([CH, 1], F32, tag="b2h")
            bout = per.tile([CH, 2], F32, tag="bout")
            boutrow = per.tile([1, CIN], BF16, tag="boutrow")
            maskC = per.tile([128, 384], BF16, tag="maskC")
            maskC2g = per.tile([128, 896], BF16, tag="maskC2g")
            dvec = per.tile([64, 128], F32, tag="dvec")
            onescol = per.tile([128, 1], BF16, tag="onescol")
            ones1 = per.tile([1, 512], BF16, tag="ones1")
            ident = per.tile([128, 128], BF16, tag="ident")
            denq = per.tile([64, 128], F32, tag="denq")
            denqb = per.tile([64, 128], BF16, tag="denqb")
            newt = per.tile([64, 128], F32, tag="newt")
            ntmp = per.tile([64, 128], F32, tag="ntmp")

            nc.sync.dma_start(w1t0[:], w1t_d[0:128, :])
            nc.sync.dma_start(w1t1[:], w1t_d[128:256, :])
            nc.scalar.dma_start(b1[:], b1_d[:])
            nc.vector.memset(hpad[:, 0:128], 0.0)
            nc.vector.memset(maskC2g[:, 384:512], 0.0)
            nc.vector.memset(hpad[:, PADPOS - 128:PADPOS], 0.0)

            for _rep in range(repeat):
                # ---- P1: conv1 (x cast to bf16 during DMA)
                xts = []
                for t in range(4):
                    x0 = xbp.tile([128, 1024], BF16, tag="x0")
                    x1 = xbp.tile([128, 1024], BF16, tag="x1")
                    cs = slice(1024 * t, 1024 * (t + 1))
                    nc.sync.dma_start(x0[:], x_d[0:128, cs])
                    nc.scalar.dma_start(x1[:], x_d[128:256, cs])
                    xts.append((x0, x1))
                nc.scalar.dma_start(ident[:], ident_d[:])
                nc.scalar.dma_start(maskC[:], mask_d[:])
                nc.scalar.dma_start(maskC2g[:, 0:384], mask_d[:])
                nc.scalar.dma_start(maskC2g[:, 512:896], mask_d[:])
                nc.scalar.dma_start(onescol[:], onescol_d[:])
                nc.scalar.dma_start(w2t[:], w2t_d[:])
                nc.scalar.dma_start(b2h[:], b2h_d[:])
                nc.sync.dma_start(dvec[:], dvec_d[:])
                nc.sync.dma_start(ones1[:], ones1_d[:])
                nc.sync.dma_start(wot[:], wot_d[:])
                nc.sync.dma_start(bout[:], bout_d[:])
                nc.sync.dma_start(boutrow[:], boutr_d[:])
                for t in range(4):
                    x0, x1 = xts[t]
                    for u in range(2):
                        ps = pss.tile([128, 512], F32, tag="ps")
                        sl = slice(512 * u, 512 * (u + 1))
                        nc.tensor.matmul(ps[:], w1t0[:], x0[:, sl], start=True, stop=False)
                        nc.tensor.matmul(ps[:], w1t1[:], x1[:, sl], start=False, stop=True)
                        o = 128 + 1024 * t + 512 * u
                        nc.vector.tensor_scalar(
                            out=hpad[:, o:o + 512], in0=ps[:],
                            scalar1=b1[:], scalar2=None, op0=ALU.add)

                # ---- P2: hT via PE transposes (bf16 psum), evac on DVE/ACT
                for c4 in range(9):
                    pt = pss.tile([128, 512], BF16, tag="ps", name=f"pt{c4}")
                    n4 = min(4, NCHUNK - 4 * c4)
                    for k4 in range(n4):
                        c = 4 * c4 + k4
                        nc.tensor.transpose(pt[:, 128 * k4:128 * (k4 + 1)],
                                            hpad[:, 128 * c:128 * (c + 1)], ident[:])
                    nc.vector.tensor_copy(hT[:, 512 * c4:512 * c4 + 128 * n4],
                                          pt[:, 0:128 * n4])

                # ---- P3a: scores/exp/mask per chunk pair + inline denominators
                dn = None
                for cp in range(NCHUNK // 2):
                    sc = psc.tile([128, 1024], F32, tag="sc", name=f"sc{cp}")
                    spans = []
                    for ci in range(2):
                        c = 2 * cp + ci
                        lo, hi = max(0, c - 2), min(NSUB - 1, c)
                        nsubs = hi - lo + 1
                        alo = lo - (c - 2)
                        spans.append((alo, alo + nsubs))
                        dst = sc[:, 512 * ci + 128 * alo:512 * ci + 128 * (alo + nsubs)]
                        nc.tensor.matmul(
                            dst, hpad[:, 128 * c:128 * (c + 1)],
                            hpad[:, 128 * (lo + 1):128 * (hi + 2)],
                            start=True, stop=True)
                    if not sim_safe and spans == [(0, 3), (0, 3)]:
                        asl = attnm[:, 1024 * cp:1024 * cp + 896]
                        nc.scalar.activation(asl, sc[:, 0:896], AF.Exp,
                                             scale=float(SCALE))
                        nc.vector.tensor_tensor(out=asl, in0=asl,
                                                in1=maskC2g[:], op=ALU.mult)
                    else:
                        for ci, (a0, a1) in enumerate(spans):
                            ss = slice(512 * ci + 128 * a0, 512 * ci + 128 * a1)
                            asl = attnm[:, 1024 * cp + ss.start:1024 * cp + ss.stop]
                            nc.scalar.activation(asl, sc[:, ss], AF.Exp,
                                                 scale=float(SCALE))
                            nc.vector.tensor_tensor(
                                out=asl, in0=asl,
                                in1=maskC[:, 128 * a0:128 * a1], op=ALU.mult)
                    # denominators: 4-row super-sub S ready once chunk 2S+3 done
                    for S in ({cp - 1} if cp >= 1 else set()):
                        if not 0 <= S < 16:
                            continue
                        if S % 2 == 0 or dn is None:
                            dn = psd.tile([1, 512], F32, tag="dn", name=f"dn{S}")
                        base = 256 * (S % 2)
                        for h2 in range(2):
                            s2 = 2 * S + h2
                            for j in range(3):
                                c = s2 + j
                                aa = 2 - j
                                nc.tensor.matmul(
                                    dn[0:1, base + 128 * h2:base + 128 * (h2 + 1)],
                                    onescol[:],
                                    attnm[:, 512 * c + 128 * aa:512 * c + 128 * (aa + 1)],
                                    start=(j == 0), stop=(j == 2))
                        if S % 2 == 1:
                            g = S // 2
                            nc.vector.tensor_copy(
                                denrow[0:1, 512 * g:512 * (g + 1)], dn[0:1, :])

                # ---- P3c: conv2 + tanh -> Pg = relu(tanh)+1 (gate pre recip)
                for t in range(8):
                    pz = pss.tile([128, 512], F32, tag="ps")
                    nc.tensor.matmul(pz[:], w2t[:],
                                     hpad[:, 128 + 512 * t:128 + 512 * (t + 1)],
                                     start=True, stop=True)
                    tg = smp.tile([128, 512], BF16, tag="tg")
                    nc.scalar.activation(tg[:], pz[:], AF.Tanh, scale=0.5, bias=b2h[:])
                    nc.vector.tensor_scalar(
                        out=Pg[:, 512 * t:512 * (t + 1)], in0=tg[:],
                        scalar1=0.0, scalar2=1.0, op0=ALU.max, op1=ALU.add)

                # ---- P3e(i): sample matmuls for groups 0-3 (PE runway)
                sp_tiles = {}

                def emit_sample_mms(g8):
                    pool = psc if g8 % 2 == 0 else pss
                    tg_ = "sc" if pool is psc else "ps"
                    sp = pool.tile([128, 512], F32, tag=tg_, name=f"sp{g8}")
                    sp_tiles[g8] = sp
                    for a4 in range(4):
                        s4 = 4 * g8 + a4
                        for j in range(3):
                            c = s4 + j
                            aa = 2 - j
                            nc.tensor.matmul(
                                sp[:, 128 * a4:128 * (a4 + 1)],
                                hT[:, 128 * c:128 * (c + 1)],
                                attnm[:, 512 * c + 128 * aa:512 * c + 128 * (aa + 1)],
                                start=(j == 0), stop=(j == 2))

                def emit_attr_convout(g8):
                    sp = sp_tiles.pop(g8)
                    gsl = slice(512 * g8, 512 * (g8 + 1))
                    nc.vector.tensor_tensor(out=attr[:, gsl], in0=sp[:],
                                            in1=Pg[:, gsl], op=ALU.mult)
                    for oc in range(2):
                        po = pss.tile([128, 512], F32, tag="ps", name=f"po{g8}_{oc}")
                        if oc == 1:
                            nc.tensor.matmul(po[:], boutrow[0:1, 128:256],
                                             ones1[0:1, :], start=True, stop=False)
                            nc.tensor.matmul(po[:], wot[:, 128:256], attr[:, gsl],
                                             start=False, stop=True)
                        else:
                            nc.tensor.matmul(po[:], wot[:, 0:128], attr[:, gsl],
                                             start=True, stop=True)
                        ot = otp.tile([128, 512], F32, tag="ot")
                        if oc == 1:
                            nc.scalar.activation(ot[:], po[:], AF.Copy)
                        else:
                            nc.vector.tensor_scalar(out=ot[:], in0=po[:],
                                                    scalar1=bout[:, 0:1],
                                                    scalar2=None, op0=ALU.add)
                        nc.sync.dma_start(out_d[128 * oc:128 * (oc + 1), gsl], ot[:])


                def emit_chain_half(hh):
                    hs = slice(32 * hh, 32 * hh + 16)
                    rs = slice(2048 * hh, 2048 * (hh + 1))
                    nc.sync.dma_start(
                        denqb[hs, :],
                        denrow[0:1, rs].rearrange("o (s f) -> o s f", s=16))
                    nc.vector.tensor_copy(denq[hs, :], denqb[hs, :])
                    nc.vector.tensor_tensor(out=denq[hs, :], in0=denq[hs, :],
                                            in1=dvec[hs, :], op=ALU.add)
                    nc.vector.tensor_scalar(out=newt[hs, :].bitcast(I32),
                                            in0=denq[hs, :].bitcast(I32),
                                            scalar1=0, scalar2=None, op0=ALU.bitwise_not)
                    nc.vector.tensor_scalar(out=newt[hs, :].bitcast(I32),
                                            in0=newt[hs, :].bitcast(I32),
                                            scalar1=RECIP_MAGIC + 1, scalar2=None, op0=ALU.add)
                    for _ in range(3):
                        nc.vector.tensor_tensor(out=ntmp[hs, :], in0=denq[hs, :],
                                                in1=newt[hs, :], op=ALU.mult)
                        nc.vector.tensor_scalar(out=ntmp[hs, :], in0=ntmp[hs, :],
                                                scalar1=-1.0, scalar2=2.0,
                                                op0=ALU.mult, op1=ALU.add)
                        nc.vector.tensor_tensor(out=newt[hs, :], in0=newt[hs, :],
                                                in1=ntmp[hs, :], op=ALU.mult)
                    nc.vector.tensor_scalar(out=denqb[hs, :], in0=newt[hs, :],
                                            scalar1=0.5, scalar2=None, op0=ALU.mult)
                    nc.sync.dma_start(
                        recrow[0:1, rs].rearrange("o (s f) -> o s f", s=16),
                        denqb[hs, :])
                    for t in range(4 * hh, 4 * hh + 4):
                        if t % 2 == 0:
                            pb = psd.tile([128, 512], F32, tag="dn", name=f"pb{t}")
                        else:
                            pb = pss.tile([128, 512], F32, tag="ps", name=f"pb{t}")
                        nc.tensor.matmul(pb[0:128, :], ones1[0:1, 0:128],
                                         recrow[0:1, 512 * t:512 * (t + 1)],
                                         start=True, stop=True)
                        sl = slice(512 * t, 512 * (t + 1))
                        nc.vector.tensor_tensor(out=Pg[:, sl], in0=Pg[:, sl],
                                                in1=pb[0:128, :], op=ALU.mult)

                emit_chain_half(0)
                for g8 in range(4):
                    emit_sample_mms(g8)
                for g8 in range(4):
                    emit_attr_convout(g8)
                for g8 in range(4, 6):
                    emit_sample_mms(g8)
                emit_chain_half(1)
                for g8 in range(6, 8):
                    emit_sample_mms(g8)
                for g8 in range(4, 8):
                    emit_attr_convout(g8)


    return nc


def _prep_inputs(x, W1, b1, W2, b2, Wout, bout):
    maskC, D = _build_mask_and_D()
    bf = ml_dtypes.bfloat16
    common = {
        "W1T": np.ascontiguousarray(W1.T).astype(bf),
        "b1": np.asarray(b1, np.float32).reshape(CH, 1),
        "W2T": np.ascontiguousarray(W2.T).astype(bf),
        "b2h": (0.5 * np.asarray(b2, np.float32)).reshape(CH, 1),
        "WoutT": np.ascontiguousarray(Wout.T).astype(bf),
        "bout2": np.ascontiguousarray(np.asarray(bout, np.float32).reshape(2, CH).T),
        "boutrow": np.asarray(bout, np.float32).reshape(1, CIN).astype(bf),
        "maskC": maskC,
        "Dvec": D,
        "ident": np.eye(128, dtype=np.float32).astype(bf),
        "onescol_c": np.ones((128, 1), np.float32).astype(bf),
        "ones1_c": np.ones((1, 512), np.float32).astype(bf),
    }
    in_maps = []
    for i in range(N):
        m = dict(common)
        m["x"] = np.ascontiguousarray(
            np.asarray(x[i], np.float32).reshape(CIN, HW)).astype(bf)
        in_maps.append(m)
    return in_maps


_CACHED = {}


def kernel(x, W1, b1, W2, b2, Wout, bout):
    if "nc" not in _CACHED:
        nc = build_nc()
        nc.finalize()
        _CACHED["nc"] = nc
    nc = _CACHED["nc"]
    in_maps = _prep_inputs(x, W1, b1, W2, b2, Wout, bout)
    res = run_bass_kernel_spmd(nc, in_maps, core_ids=list(range(N)))
    out = np.stack([res.results[i]["out"].reshape(CIN, H, W) for i in range(N)])
    return out.astype(np.float32)

